# revision 10
# baseline (speedup 1.0000x reference)
"""Trainium2 Bass kernel for nn_ChannelizedLinearCompression.

Computation (fp32 reference):
    h1      = relu(einsum('bcn,cnh->bch', x, W1) + b1)   # [B, C, H]
    h2      = relu(einsum('bch,chk->bck', h1, W2) + b2)  # [B, C, 10]
    scalars = einsum('bck,ck->bc', h2, W3) + b3          # [B, C]
    out     = relu(scalars @ Wf1 + bf1) @ Wf2 + bf2      # [B, 16]

Sharding: 2 batch groups x 4 channel groups over 8 cores. Each core gets
x^T[c_loc, N, b_loc] (host-transposed so every big DMA is contiguous) and
computes scalars^T[c_loc, b_loc] on device; the tiny final MLP (0.003% of
the FLOPs) runs on host.

Stage 1 (99.9% of FLOPs) runs in fp8-e4m3 with DoubleRow perf mode: the
PE holds 2 fp8 weights per cell (K=256 per instruction) and does 2
multiplies/cycle -> ~1.77x bf16 ALU rate, plus half the HBM bytes of
fp16. Host pre-scales x by 2^4 and W1 by 2^10 to keep the operands out
of e4m3's subnormal range; the 2^-14 descale folds into the stage-1
activation's scale. Numpy sim of this quantization gives final rel err
~7e-4 vs the 2e-2 gate.

DoubleRow layout per matmul (ISA: 3D APs [Ki=128, Ko=2, dim], Ko stride
16B-aligned -> W tile free dim padded 286->288):
  lhsT = W1 pair  [128, 2, hs<=128]  (two 128-deep k-tiles of W1[c])
  rhs  = x pair   [128, 2, 512]      (same two k-tiles of xT[c])
  out  = psum     [hs, 512]          accumulated over the 32 k-pairs
"""

import os
from contextlib import ExitStack

import numpy as np
import ml_dtypes

import concourse.bass as bass
import concourse.tile as tile
from concourse import bacc, mybir
from concourse.bass_utils import run_bass_kernel_spmd
from concourse._compat import get_trn_type

# Problem shapes (hardcoded; kernel.py must be self-contained).
B, C, N = 2048, 12, 8192
H, MID = 286, 10
FINAL_HIDDEN, LOWDIM = 30, 16
BG, CG = 2, 4  # batch groups x channel groups = 8 cores
B_LOC, C_LOC = B // BG, C // CG
HP = 288  # H padded so the DoubleRow k-pair stride is 16B-aligned

F32 = mybir.dt.float32
F16 = mybir.dt.float16
FP8 = mybir.dt.float8e4
NP_FP8 = ml_dtypes.float8_e4m3  # TRN e4m3: max normal 240
RELU = mybir.ActivationFunctionType.Relu
IDENT = mybir.ActivationFunctionType.Identity
DR = mybir.MatmulPerfMode.DoubleRow

# Pre-scales keep x (std 1) and W1 (std 0.02) in e4m3's normal range.
SX, SW = 16.0, 1024.0
DESCALE = 1.0 / (SX * SW)

HCH = [(0, 128), (128, 128), (256, 30)]  # stage-1/2 h-chunks

LAST = {}  # introspection for test.py (exec_time_ns etc.); harness ignores


def build_nc(b_loc=B_LOC, c_loc=C_LOC, n=N):
    assert n % 256 == 0 and b_loc % 512 == 0
    nt = n // 256  # k-pairs
    nj = b_loc // 512

    nc = bacc.Bacc(get_trn_type() or "TRN2", target_bir_lowering=False)
    xt = nc.declare_dram_parameter("xt", [c_loc, n, b_loc], FP8, isOutput=False)
    w1 = nc.declare_dram_parameter("w1", [c_loc, n, H], FP8, isOutput=False)
    b1 = nc.declare_dram_parameter("b1", [c_loc, H, 1], F32, isOutput=False)
    w2 = nc.declare_dram_parameter("w2", [c_loc, H, MID], F16, isOutput=False)
    b2 = nc.declare_dram_parameter("b2", [c_loc, MID, 1], F32, isOutput=False)
    w3 = nc.declare_dram_parameter("w3", [c_loc, MID, 1], F16, isOutput=False)
    b3 = nc.declare_dram_parameter("b3", [c_loc, 1, 1], F32, isOutput=False)
    out = nc.declare_dram_parameter("out", [c_loc, b_loc], F32, isOutput=True)

    with tile.TileContext(nc) as tc, ExitStack() as ctx:
        xp = ctx.enter_context(tc.tile_pool(name="xp", bufs=14))
        wp = ctx.enter_context(tc.tile_pool(name="wp", bufs=14))
        hp = ctx.enter_context(tc.tile_pool(name="hp", bufs=6))
        sp = ctx.enter_context(tc.tile_pool(name="sp", bufs=30))
        op = ctx.enter_context(tc.tile_pool(name="op", bufs=4))
        pp = ctx.enter_context(
            tc.tile_pool(name="pp", bufs=8, space=bass.MemorySpace.PSUM)
        )

        def stage1(c):
            """Emit channel c's stage-1 t-loop + small-weight DMAs."""
            # stage 1: h1T[h, b] = relu((W1[c].T @ xT[c]) * 2^-14 + b1[c])
            ps = [[pp.tile([128, 512], F32, tag="ps", name=f"ps{c}_{i}_{j}")
                   for j in range(nj)] for i in range(len(HCH))]
            for t in range(nt):
                xt2 = xp.tile([128, 2, b_loc], FP8, tag="xt2",
                              name=f"xt2_{c}_{t}")
                w12 = wp.tile([128, 2, HP], FP8, tag="w12", name=f"w12_{c}_{t}")
                # One merged 3D DMA per operand pair. Issue engines are
                # chosen so no sequencer is on the critical path: the Sync
                # sequencer's ~565ns DGE config per dma_start would gate the
                # t-loop if all transfers went through it, and ScalarE must
                # stay free for the psum-drain activations (it only helps
                # with channel 0's x, where the cold DMA pipeline would
                # otherwise starve the PE).
                k0 = t * 256
                xeng = nc.scalar if (c == 0 and t % 2 == 1) else nc.sync
                xeng.dma_start(
                    xt2[:, :, :],
                    xt[c, k0:k0 + 256, :].rearrange("(i p) b -> p i b", i=2))
                nc.gpsimd.dma_start(
                    w12[:, :, 0:H],
                    w1[c, k0:k0 + 256, :].rearrange("(i p) h -> p i h", i=2))
                for i, (h0, hs) in enumerate(HCH):
                    for j in range(nj):
                        nc.tensor.matmul(
                            ps[i][j][:hs, :],
                            w12[:, :, h0:h0 + hs],
                            xt2[:, :, j * 512:(j + 1) * 512],
                            start=(t == 0),
                            stop=(t == nt - 1),
                            perf_mode=DR,
                        )
            # Small weights for the tail stages, emitted after the t-loop so
            # they don't delay the first x-tile configs on the Sync engine;
            # they are only needed one channel-window later.
            b1t = [sp.tile([hs, 1], F32, tag="b1t", name=f"b1t{c}_{i}")
                   for i, (h0, hs) in enumerate(HCH)]
            w2t = [sp.tile([hs, MID], F16, tag="w2t", name=f"w2t{c}_{i}")
                   for i, (h0, hs) in enumerate(HCH)]
            for i, (h0, hs) in enumerate(HCH):
                nc.sync.dma_start(b1t[i][:, :], b1[c, h0:h0 + hs, :])
                nc.sync.dma_start(w2t[i][:, :], w2[c, h0:h0 + hs, :])
            w3t = sp.tile([MID, 1], F16, tag="w3t", name=f"w3t{c}")
            b2t = sp.tile([MID, 1], F32, tag="b2t", name=f"b2t{c}")
            b3t = sp.tile([1, 1], F32, tag="b3t", name=f"b3t{c}")
            nc.sync.dma_start(w3t[:, :], w3[c])
            nc.sync.dma_start(b2t[:, :], b2[c])
            nc.sync.dma_start(b3t[:, :], b3[c])
            return ps, b1t, w2t, w3t, b2t, b3t

        def tail(c, st):
            """Emit channel c's psum drain + stages 2/3 + output DMA.

            Fully per-j ordered: j=0's drain -> stage2 -> stage3 chain is
            emitted before j=1's drains, so the PE works on j=0's tiny
            matmuls while ScalarE drains j=1 (matters for the last channel,
            whose tail is not hidden behind another channel's stage 1).
            """
            ps, b1t, w2t, w3t, b2t, b3t = st
            h1t = [hp.tile([hs, b_loc], F16, tag="h1t", name=f"h1t{c}_{i}")
                   for i, (h0, hs) in enumerate(HCH)]
            p2 = [pp.tile([MID, 512], F32, tag="ps", name=f"p2{c}_{j}")
                  for j in range(nj)]
            p3 = [pp.tile([1, 512], F32, tag="ps", name=f"p3{c}_{j}")
                  for j in range(nj)]
            h2t = op.tile([MID, b_loc], F16, tag="h2t", name=f"h2t{c}")
            sct = op.tile([1, b_loc], F32, tag="sct", name=f"sct{c}")
            for j in range(nj):
                sl = slice(j * 512, (j + 1) * 512)
                # drain psums -> h1 (fp16, h-major in chunks of 128/128/30)
                for i, (h0, hs) in enumerate(HCH):
                    nc.scalar.activation(
                        h1t[i][:, sl], ps[i][j][:hs, :],
                        RELU, bias=b1t[i][:, :], scale=DESCALE,
                    )
                # stage 2: h2T[k, b] = relu(W2[c].T @ h1T + b2[c])
                for i, (h0, hs) in enumerate(HCH):
                    nc.tensor.matmul(
                        p2[j][:, :], w2t[i][:, :], h1t[i][:, sl],
                        start=(i == 0), stop=(i == len(HCH) - 1),
                    )
                nc.scalar.activation(h2t[:, sl], p2[j][:, :], RELU,
                                     bias=b2t[:, :])
                # stage 3: scalarsT[c, b] = W3[c].T @ h2T + b3[c]
                nc.tensor.matmul(p3[j][:, :], w3t[:, :], h2t[:, sl],
                                 start=True, stop=True)
                nc.scalar.activation(sct[:, sl], p3[j][:, :], IDENT,
                                     bias=b3t[:, :])
            nc.sync.dma_start(out[c:c + 1, :], sct[0:1, :])

        # Software-pipelined emission: channel c's tail (drain + tiny
        # stages 2/3) is emitted AFTER channel c+1's stage-1 matmuls, so the
        # in-order PE never idles waiting on ScalarE drains at channel
        # boundaries. PSUM rotation stays legal: c+1's first psum group gets
        # the 2 spare banks; later groups reuse banks as c's drains retire.
        st = [None] * c_loc
        st[0] = stage1(0)
        for c in range(1, c_loc):
            st[c] = stage1(c)
            tail(c - 1, st[c - 1])
        tail(c_loc - 1, st[c_loc - 1])

    nc.compile()
    return nc


_NC_CACHE = {}


def _get_nc():
    key = (B_LOC, C_LOC, N)
    if key not in _NC_CACHE:
        _NC_CACHE[key] = build_nc()
    return _NC_CACHE[key]


def _to_fp8(arr, scale):
    """fp32 ndarray -> e4m3 with pre-scale, clipped to TRN's +-240 range."""
    return np.clip(arr * scale, -240.0, 240.0).astype(NP_FP8)


def _transpose_shard_u8(xs):
    """[b_loc, c_loc, n] byte tensor -> contiguous [c_loc, n, b_loc]."""
    u8 = np.ascontiguousarray(xs).view(np.uint8)
    try:
        import torch
        try:
            torch.set_num_threads(max(os.cpu_count() or 1, 1))
        except Exception:
            pass
        res = torch.from_numpy(u8).permute(1, 2, 0).contiguous().numpy()
    except ImportError:
        res = np.ascontiguousarray(np.transpose(u8, (1, 2, 0)))
    return res.view(NP_FP8)


def kernel(x, W1, b1, W2, b2, W3, b3, Wf1, bf1, Wf2, bf2):
    x = np.asarray(x, dtype=np.float32)
    W1 = np.asarray(W1, dtype=np.float32)
    b1 = np.asarray(b1, dtype=np.float32)
    W2 = np.asarray(W2, dtype=np.float32)
    b2 = np.asarray(b2, dtype=np.float32)
    W3 = np.asarray(W3, dtype=np.float32)
    b3 = np.asarray(b3, dtype=np.float32)

    nc = _get_nc()

    # cast to 1-byte fp8 before transposing so the shuffle moves 1/4 the bytes
    x8 = _to_fp8(x, SX)

    in_maps = []
    for ib in range(BG):
        bs = slice(ib * B_LOC, (ib + 1) * B_LOC)
        for ic in range(CG):
            cs = slice(ic * C_LOC, (ic + 1) * C_LOC)
            in_maps.append({
                "xt": _transpose_shard_u8(x8[bs, cs, :]),
                "w1": _to_fp8(W1[cs], SW),
                "b1": np.ascontiguousarray(b1[cs])[:, :, None],
                "w2": np.ascontiguousarray(W2[cs], dtype=np.float16),
                "b2": np.ascontiguousarray(b2[cs])[:, :, None],
                "w3": np.ascontiguousarray(W3[cs], dtype=np.float16)[:, :, None],
                "b3": np.ascontiguousarray(b3[cs])[:, None, None],
            })

    res = run_bass_kernel_spmd(nc, in_maps, list(range(BG * CG)))
    LAST["exec_time_ns"] = res.exec_time_ns
    LAST["results"] = res

    scalars = np.empty((B, C), np.float32)
    idx = 0
    for ib in range(BG):
        bs = slice(ib * B_LOC, (ib + 1) * B_LOC)
        for ic in range(CG):
            cs = slice(ic * C_LOC, (ic + 1) * C_LOC)
            scalars[bs, cs] = res.results[idx]["out"].T
            idx += 1

    # Final tiny MLP (C -> 30 -> lowdim) on host in fp32.
    h = np.maximum(scalars @ np.asarray(Wf1, np.float32)
                   + np.asarray(bf1, np.float32), 0.0)
    return (h @ np.asarray(Wf2, np.float32)
            + np.asarray(bf2, np.float32)).astype(np.float32)
